# revision 6
# baseline (speedup 1.0000x reference)
"""MultiHeadAttention (rotary + masked softmax + fc + residual + layernorm)
Bass/Tile kernel for 8 Trainium2 NeuronCores.

Sharding: tensor-parallel on heads. 16 heads / 8 cores = 2 heads per core.
Each core:
  - projects q/k/v onto its 2 heads' weight slices (contraction over full DM)
  - applies the rotary ("Angle") encoding to its q/k projections
  - computes masked softmax attention for its (2 heads x 2 batches)
  - writes its slice of the [H*B, L, L] attention tensor
  - computes a partial fc output (row-parallel over the 128 av-dims it owns)
Host: transposes inputs once, sums the 8 fc partials, adds bias + residual,
applies the final layernorm.

Matmuls run as float32r (full-rate fp32 streaming on the PE array) when
use_f32r; the attention-probability path (p, its transposes, the attn output)
stays plain f32.
"""

import numpy as np

# Problem dims (hardcoded per the harness contract)
H, DM, DK, DV = 16, 1024, 64, 64
B, L = 2, 2048
TEMP = float(np.sqrt(DK))
EPS = 1e-5
NCORES = 8
H_LOC = H // NCORES          # heads per core
D_LOC = H_LOC * DK           # 128 proj dims per core
P = 128
NEG = -1.0e30

_CACHE = {}


# ----------------------------------------------------------------------------
# Device kernel (per-core SPMD program)
# ----------------------------------------------------------------------------

def _emit(tc, nc, t, L_, B_, use_f32r=True):
    """Emit the per-core program. `t` maps name -> dram AP."""
    from concourse import mybir

    f32 = mybir.dt.float32
    fmm = mybir.dt.float32r if use_f32r else f32
    Alu = mybir.AluOpType
    Act = mybir.ActivationFunctionType

    N_ = B_ * L_                 # total tokens
    TG = N_ // 512               # 512-token groups for projections
    DMK = DM // P                # 8 k-tiles over the DM contraction
    KT = L_ // P                 # 128-wide k tiles per batch
    QG = max(L_ // 512, 1)       # 512-row q groups per batch
    QT_G = min(L_, 512) // P     # q tiles per group
    KC = max(L_ // 512, 1)       # 512-wide k chunks
    KCW = min(L_, 512)           # k chunk width

    with tc.tile_pool(name="persist", bufs=1) as persist:
        qT = persist.tile([P, N_], fmm)      # rotated q projection [dims, tok]
        kT = persist.tile([P, N_], fmm)      # rotated k projection [dims, tok]
        vnat = persist.tile([P, N_], fmm)    # v proj, natural: (b,kt) blocks of [tok128, dims128]
        avT = persist.tile([P, N_], fmm)     # attention output [dims, tok]
        wfct_sb = persist.tile([P, DM], fmm)
        ident_sb = persist.tile([P, P], f32)
        nc.sync.dma_start(out=wfct_sb, in_=t["wfcT"])
        nc.sync.dma_start(out=ident_sb, in_=t["ident"])

        # ---- Phase A+B: q/k/v projections, rotary on q/k, v transpose ----
        with (
            tc.tile_pool(name="wpool", bufs=1) as wp,
            tc.tile_pool(name="xpool", bufs=3) as xp,
            tc.tile_pool(name="rotpool", bufs=3) as rp,
            tc.tile_pool(name="ppsum", bufs=2, space="PSUM") as ppp,
            tc.tile_pool(name="spsum", bufs=2, space="PSUM") as swp_ps,
            tc.tile_pool(name="vpsum", bufs=2, space="PSUM") as vps,
        ):
            w_sbs = {}
            b_sbs = {}
            for nm in ("q", "k", "v"):
                w_sb = wp.tile([P, DMK, P], fmm, tag=f"w{nm}")
                nc.sync.dma_start(
                    out=w_sb, in_=t[f"w{nm}T"].rearrange("(t p) d -> p t d", p=P)
                )
                w_sbs[nm] = w_sb
                b_sb = wp.tile([P, 1], f32, tag=f"b{nm}")
                nc.sync.dma_start(out=b_sb, in_=t[f"b{nm}"])
                b_sbs[nm] = b_sb
            pswap_sb = wp.tile([P, P], fmm, tag="pswap")
            nc.sync.dma_start(out=pswap_sb, in_=t["pswap"])
            cosA_sb = wp.tile([P, N_], f32, tag="cosA")
            nc.sync.dma_start(out=cosA_sb, in_=t["cosA"])
            sinS_sb = wp.tile([P, N_], f32, tag="sinS")
            nc.sync.dma_start(out=sinS_sb, in_=t["sinS"])

            projv_sb = wp.tile([P, N_], fmm, tag="projv")

            for nm, dst, rot in (("q", qT, True), ("k", kT, True), ("v", projv_sb, False)):
                xT_ap = t[f"x{nm}T"]
                w_sb = w_sbs[nm]
                for tg in range(TG):
                    ps = ppp.tile([P, 512], f32, tag="projps")
                    for kt in range(DMK):
                        x_sb = xp.tile([P, 512], fmm, tag="xin")
                        nc.sync.dma_start(
                            out=x_sb,
                            in_=xT_ap[kt * P:(kt + 1) * P, tg * 512:(tg + 1) * 512],
                        )
                        nc.tensor.matmul(
                            ps, w_sb[:, kt, :], x_sb,
                            start=(kt == 0), stop=(kt == DMK - 1),
                        )
                    sl = slice(tg * 512, (tg + 1) * 512)
                    # psum -> sbuf copy, adding the projection bias
                    nc.scalar.activation(dst[:, sl], ps, Act.Identity, bias=b_sbs[nm])
                    if rot:
                        # rotary: out = x*cos' + swap(x)*sin'  (swap via PE perm)
                        swp = swp_ps.tile([P, 512], f32, tag="swp")
                        nc.tensor.matmul(swp, pswap_sb, dst[:, sl],
                                         start=True, stop=True)
                        t1 = rp.tile([P, 512], f32, tag="rot1")
                        nc.vector.tensor_tensor(t1, dst[:, sl], cosA_sb[:, sl], Alu.mult)
                        t2 = rp.tile([P, 512], f32, tag="rot2")
                        nc.vector.tensor_tensor(t2, swp, sinS_sb[:, sl], Alu.mult)
                        nc.vector.tensor_tensor(dst[:, sl], t1, t2, Alu.add)

            # v transpose into natural [tok, dims] blocks (f32 transpose mode)
            for b in range(B_):
                for kt in range(KT):
                    col = b * L_ + kt * P
                    tps = vps.tile([P, P], f32, tag="vtps")
                    nc.tensor.transpose(
                        tps, projv_sb[:, col:col + P].bitcast(f32), ident_sb
                    )
                    nc.scalar.copy(vnat[:, (b * KT + kt) * P:(b * KT + kt + 1) * P], tps)

        # ---- Phase D: attention ----
        with (
            tc.tile_pool(name="mpool", bufs=1) as mp,
            tc.tile_pool(name="tpool", bufs=1) as tp,
            tc.tile_pool(name="wkpool", bufs=2) as wk,
            tc.tile_pool(name="probp", bufs=2) as prp,
            tc.tile_pool(name="denp", bufs=8) as dnp,
            tc.tile_pool(name="scps", bufs=1, space="PSUM") as scps,
            tc.tile_pool(name="ptps", bufs=2, space="PSUM") as ptps,
            tc.tile_pool(name="avps", bufs=1, space="PSUM") as avps,
        ):
            for b in range(B_):
                for qg in range(QG):
                    mneg_sb = mp.tile([P, QT_G, L_], f32, tag="mneg")
                    nc.sync.dma_start(
                        out=mneg_sb,
                        in_=t["mneg"][b, qg * QT_G * P:(qg + 1) * QT_G * P, :]
                        .rearrange("(t p) k -> p t k", p=P),
                    )
                    for h in range(H_LOC):
                        hd = slice(h * DK, (h + 1) * DK)
                        pT_sb = tp.tile([P, KT, QT_G * P], fmm, tag="pT")
                        for qt in range(QT_G):
                            q0 = b * L_ + (qg * QT_G + qt) * P
                            sps = scps.tile([P, L_], f32, tag="scores")
                            for kc in range(KC):
                                nc.tensor.matmul(
                                    sps[:, kc * KCW:(kc + 1) * KCW],
                                    qT[hd, q0:q0 + P],
                                    kT[hd, b * L_ + kc * KCW: b * L_ + (kc + 1) * KCW],
                                    start=True, stop=True,
                                )
                            sm = wk.tile([P, L_], f32, tag="sm")
                            nc.vector.tensor_tensor(sm, sps, mneg_sb[:, qt, :], Alu.add)
                            e = wk.tile([P, L_], f32, tag="e")
                            den = dnp.tile([P, 1], f32, tag="den")
                            nc.scalar.activation(e, sm, Act.Exp, scale=1.0 / TEMP,
                                                 accum_out=den)
                            rden = dnp.tile([P, 1], f32, tag="rden")
                            nc.vector.reciprocal(rden, den)
                            p = prp.tile([P, L_], f32, tag="p")
                            nc.gpsimd.tensor_scalar_mul(p, e, rden)
                            qrow = (qg * QT_G + qt) * P
                            nc.sync.dma_start(
                                out=t["attn_o"][h, b, qrow:qrow + P, :], in_=p
                            )
                            # transpose p into [k, q] strips for the av matmul
                            for ktg in range((KT + 3) // 4):
                                n_in = min(4, KT - ktg * 4)
                                tps = ptps.tile([P, 512], f32, tag="ptps")
                                for j in range(n_in):
                                    kt = ktg * 4 + j
                                    nc.tensor.transpose(
                                        tps[:, j * P:(j + 1) * P],
                                        p[:, kt * P:(kt + 1) * P],
                                        ident_sb,
                                    )
                                nc.vector.tensor_copy(
                                    out=pT_sb[:, ktg * 4:ktg * 4 + n_in, qt * P:(qt + 1) * P],
                                    in_=tps[:, :n_in * P].rearrange("p (t q) -> p t q", q=P),
                                )
                        # av^T = v^T @ p^T for this (h, b, qg)
                        aps = avps.tile([DV, QT_G * P], f32, tag="av")
                        for kt in range(KT):
                            nc.tensor.matmul(
                                aps,
                                vnat[:, (b * KT + kt) * P + h * DK:
                                     (b * KT + kt) * P + (h + 1) * DK],
                                pT_sb[:, kt, :],
                                start=(kt == 0), stop=(kt == KT - 1),
                            )
                        c0 = b * L_ + qg * QT_G * P
                        nc.scalar.copy(avT[hd, c0:c0 + QT_G * P], aps)

        # ---- Phase E: partial fc ----
        with (
            tc.tile_pool(name="ypool", bufs=3) as yp,
            tc.tile_pool(name="fcps", bufs=2, space="PSUM") as fps_pool,
        ):
            for tt in range(N_ // P):
                fps = fps_pool.tile([P, DM], f32, tag="fc")
                for nk in range(DM // 512):
                    nc.tensor.matmul(
                        fps[:, nk * 512:(nk + 1) * 512],
                        avT[:, tt * P:(tt + 1) * P],
                        wfct_sb[:, nk * 512:(nk + 1) * 512],
                        start=True, stop=True,
                    )
                y_sb = yp.tile([P, DM], f32, tag="y")
                nc.scalar.copy(y_sb, fps)
                nc.sync.dma_start(out=t["y_part"][tt * P:(tt + 1) * P, :], in_=y_sb)


def build(L_=L, B_=B, use_f32r=True, compile_=True):
    import concourse.bass as bass  # noqa: F401
    from concourse import bacc, mybir, tile

    f32 = mybir.dt.float32
    fmm = mybir.dt.float32r if use_f32r else f32
    N_ = B_ * L_
    nc = bacc.Bacc("TRN2", target_bir_lowering=False, debug=False,
                   num_devices=NCORES)

    t = {}

    def inp(name, shape, dt=f32):
        t[name] = nc.dram_tensor(name, list(shape), dt, kind="ExternalInput").ap()

    def outp(name, shape, dt=f32):
        t[name] = nc.dram_tensor(name, list(shape), dt, kind="ExternalOutput").ap()

    for nm in ("q", "k", "v"):
        inp(f"x{nm}T", (DM, N_), fmm)
        inp(f"w{nm}T", (DM, D_LOC), fmm)
        inp(f"b{nm}", (D_LOC, 1))
    inp("cosA", (D_LOC, N_))
    inp("sinS", (D_LOC, N_))
    inp("mneg", (B_, L_, L_))
    inp("pswap", (P, P), fmm)
    inp("ident", (P, P))
    inp("wfcT", (D_LOC, DM), fmm)
    outp("attn_o", (H_LOC, B_, L_, L_))
    outp("y_part", (N_, DM))

    with tile.TileContext(nc) as tc:
        _emit(tc, nc, t, L_, B_, use_f32r=use_f32r)

    if compile_:
        nc.compile()
    return nc


# ----------------------------------------------------------------------------
# Host-side input prep / output assembly
# ----------------------------------------------------------------------------

def _host_prep(q, k, v, mask, w_q, b_q, w_k, b_k, w_v, b_v, w_fc, L_=L, B_=B):
    """Build the 8 per-core input maps."""
    N_ = B_ * L_
    f32 = np.float32

    xqT = np.ascontiguousarray(q.reshape(N_, DM).T.astype(f32, copy=False))
    xkT = np.ascontiguousarray(k.reshape(N_, DM).T.astype(f32, copy=False))
    xvT = np.ascontiguousarray(v.reshape(N_, DM).T.astype(f32, copy=False))
    mneg = np.where(mask, f32(NEG), f32(0.0)).astype(f32, copy=False)
    mneg = np.ascontiguousarray(mneg)

    pswap = np.zeros((P, P), f32)
    idx = np.arange(0, P, 2)
    pswap[idx, idx + 1] = 1.0
    pswap[idx + 1, idx] = 1.0
    ident = np.eye(P, dtype=f32)

    # rotary tables (full-D angle enc: inv freq indexed over all H*DK/2 pairs)
    D_full = H * DK
    pos = np.tile(np.arange(1, L_ + 1, dtype=f32), B_)          # (N,)

    in_maps = []
    for c in range(NCORES):
        rs = slice(c * D_LOC, (c + 1) * D_LOC)                  # proj dim rows
        j = np.arange(c * (D_LOC // 2), (c + 1) * (D_LOC // 2), dtype=f32)
        inv = np.power(f32(10000.0), -2.0 * j / D_full).astype(f32)
        ang = inv[:, None] * pos[None, :]                       # (64, N)
        cosA = np.repeat(np.cos(ang), 2, axis=0).astype(f32)    # (128, N)
        s = np.sin(ang).astype(f32)
        sinS = np.empty((D_LOC, N_), f32)
        sinS[0::2] = -s
        sinS[1::2] = s
        m = {
            "xqT": xqT, "xkT": xkT, "xvT": xvT, "mneg": mneg,
            "pswap": pswap, "ident": ident,
            "wqT": np.ascontiguousarray(w_q[rs].T.astype(f32, copy=False)),
            "wkT": np.ascontiguousarray(w_k[rs].T.astype(f32, copy=False)),
            "wvT": np.ascontiguousarray(w_v[rs].T.astype(f32, copy=False)),
            "bq": np.ascontiguousarray(b_q[rs].astype(f32, copy=False)).reshape(D_LOC, 1),
            "bk": np.ascontiguousarray(b_k[rs].astype(f32, copy=False)).reshape(D_LOC, 1),
            "bv": np.ascontiguousarray(b_v[rs].astype(f32, copy=False)).reshape(D_LOC, 1),
            "wfcT": np.ascontiguousarray(w_fc[:, rs].T.astype(f32, copy=False)),
            "cosA": np.ascontiguousarray(cosA),
            "sinS": np.ascontiguousarray(sinS),
        }
        in_maps.append(m)
    return in_maps


def _host_finish(results, q, b_fc, gamma, beta, L_=L, B_=B):
    N_ = B_ * L_
    f32 = np.float32
    attn = np.empty((H, B_, L_, L_), f32)
    y = np.zeros((N_, DM), f32)
    for c, res in enumerate(results):
        attn[c * H_LOC:(c + 1) * H_LOC] = res["attn_o"]
        y += res["y_part"]
    y = y + b_fc[None, :].astype(f32) + q.reshape(N_, DM).astype(f32)
    mu = y.mean(axis=-1, keepdims=True, dtype=f32)
    var = y.var(axis=-1, keepdims=True, dtype=f32)
    y = (y - mu) / np.sqrt(var + EPS) * gamma[None, :] + beta[None, :]
    return (y.reshape(B_, L_, DM).astype(f32),
            attn.reshape(H * B_, L_, L_))


def kernel(q, k, v, mask, w_q, b_q, w_k, b_k, w_v, b_v, w_fc, b_fc, gamma, beta):
    from concourse import bass_utils

    key = (L, B, True)
    if key not in _CACHE:
        _CACHE[key] = build(L, B, use_f32r=True)
    nc = _CACHE[key]

    in_maps = _host_prep(q, k, v, mask, w_q, b_q, w_k, b_k, w_v, b_v, w_fc)
    res = bass_utils.run_bass_kernel_spmd(nc, in_maps, core_ids=list(range(NCORES)))
    return _host_finish(res.results, q, b_fc, gamma, beta)


# ----------------------------------------------------------------------------
# Pure-numpy golden model (for self-tests only; harness never calls this)
# ----------------------------------------------------------------------------

def golden(q, k, v, mask, w_q, b_q, w_k, b_k, w_v, b_v, w_fc, b_fc, gamma, beta,
           L_=L, B_=B):
    def angle_enc(x):
        Bx, Lx, D = x.shape
        j = np.arange(D // 2, dtype=np.float64)
        inv = np.power(10000.0, -2.0 * j / D)
        pos = np.arange(1, Lx + 1, dtype=np.float64)
        ang = pos[:, None] * inv[None, :]
        c, s = np.cos(ang), np.sin(ang)
        xp = x.reshape(Bx, Lx, D // 2, 2)
        x0, x1 = xp[..., 0], xp[..., 1]
        out = np.stack([x0 * c - x1 * s, x1 * c + x0 * s], axis=-1)
        return out.reshape(Bx, Lx, D)

    qp = angle_enc(q @ w_q.T + b_q)
    kp = angle_enc(k @ w_k.T + b_k)
    vp = v @ w_v.T + b_v
    qh = qp.reshape(B_, L_, H, DK).transpose(2, 0, 1, 3)
    kh = kp.reshape(B_, L_, H, DK).transpose(2, 0, 1, 3)
    vh = vp.reshape(B_, L_, H, DV).transpose(2, 0, 1, 3)
    scores = np.einsum("hbqd,hbkd->hbqk", qh, kh) / TEMP
    mb = mask[None]
    e = np.exp(scores) * (~mb)
    den = e.sum(-1, keepdims=True)
    attn = e / den
    attn = np.where(mb, 0.0, attn)
    out = np.einsum("hbqk,hbkd->hbqd", attn, vh)
    out = out.transpose(1, 2, 0, 3).reshape(B_, L_, H * DV)
    out = out @ w_fc.T + b_fc
    y = out + q
    mu = y.mean(-1, keepdims=True)
    var = y.var(-1, keepdims=True)
    y = (y - mu) / np.sqrt(var + EPS) * gamma + beta
    return y.astype(np.float32), attn.reshape(H * B_, L_, L_).astype(np.float32)


# revision 15
# speedup vs baseline: 40.1682x; 40.1682x over previous
"""MultiHeadAttention (rotary + masked softmax + fc + residual + layernorm)
Bass/Tile kernel for 8 Trainium2 NeuronCores.

Sharding: tensor-parallel on heads. 16 heads / 8 cores = 2 heads per core.
Each core:
  - projects q/k/v onto its 2 heads' weight slices (contraction over full DM)
  - applies the rotary ("Angle") encoding to its q/k projections
  - computes masked softmax attention for its (2 heads x 2 batches)
  - writes its slice of the [H*B, L, L] attention tensor
  - computes a partial fc output (row-parallel over the 128 av-dims it owns)
Host: transposes inputs once, sums the 8 fc partials, adds bias + residual,
applies the final layernorm.

Matmuls run as float32r (full-rate fp32 streaming on the PE array) when
use_f32r; the attention-probability path (p, its transposes, the attn output)
stays plain f32.
"""

import numpy as np

# Problem dims (hardcoded per the harness contract)
H, DM, DK, DV = 16, 1024, 64, 64
B, L = 2, 2048
TEMP = float(np.sqrt(DK))
EPS = 1e-5
NCORES = 8
H_LOC = H // NCORES          # heads per core
D_LOC = H_LOC * DK           # 128 proj dims per core
P = 128
NEG = -1.0e30

_CACHE = {}


# ----------------------------------------------------------------------------
# Device kernel (per-core SPMD program)
# ----------------------------------------------------------------------------

def _emit(tc, nc, t, L_, B_, use_f32r=True):
    """Emit the per-core program. `t` maps name -> dram AP."""
    from concourse import mybir

    f32 = mybir.dt.float32
    bf16 = mybir.dt.bfloat16
    fmm = mybir.dt.float32r if use_f32r else f32
    Alu = mybir.AluOpType
    Act = mybir.ActivationFunctionType

    N_ = B_ * L_                 # total tokens
    TG = N_ // 512               # 512-token groups for projections
    DMK = DM // P                # 8 k-tiles over the DM contraction
    KT = L_ // P                 # 128-wide k tiles per batch
    QG = max(L_ // 512, 1)       # 512-row q groups per batch
    QT_G = min(L_, 512) // P     # q tiles per group
    KC = max(L_ // 512, 1)       # 512-wide k chunks
    KCW = min(L_, 512)           # k chunk width

    with tc.tile_pool(name="persist", bufs=1) as persist:
        qT = persist.tile([P, N_], fmm)      # rotated q projection [dims, tok]
        kT = persist.tile([P, N_], fmm)      # rotated k projection [dims, tok]
        vnat = persist.tile([P, N_], fmm)    # v proj, natural: (b,kt) blocks of [tok128, dims128]
        avT = persist.tile([P, N_], fmm)     # attention output [dims, tok]
        wfct_sb = persist.tile([P, DM], fmm)
        ident_sb = persist.tile([P, P], f32)
        nc.sync.dma_start(out=wfct_sb, in_=t["wfcT"])
        nc.sync.dma_start(out=ident_sb, in_=t["ident"])

        # ---- Phase A+B: q/k/v projections, rotary on q/k, v transpose ----
        with (
            tc.tile_pool(name="wpool", bufs=1) as wp,
            tc.tile_pool(name="xpool", bufs=3) as xp,
            tc.tile_pool(name="rotpool", bufs=3) as rp,
            tc.tile_pool(name="ppsum", bufs=2, space="PSUM") as ppp,
            tc.tile_pool(name="spsum", bufs=2, space="PSUM") as swp_ps,
            tc.tile_pool(name="vpsum", bufs=2, space="PSUM") as vps,
        ):
            w_sbs = {}
            b_sbs = {}
            for nm in ("q", "k", "v"):
                w_sb = wp.tile([P, DMK, P], fmm, tag=f"w{nm}")
                nc.sync.dma_start(
                    out=w_sb, in_=t[f"w{nm}T"].rearrange("(t p) d -> p t d", p=P)
                )
                w_sbs[nm] = w_sb
                b_sb = wp.tile([P, 1], f32, tag=f"b{nm}")
                nc.sync.dma_start(out=b_sb, in_=t[f"b{nm}"])
                b_sbs[nm] = b_sb
            pswap_sb = wp.tile([P, P], fmm, tag="pswap")
            nc.sync.dma_start(out=pswap_sb, in_=t["pswap"])
            cosA_sb = wp.tile([P, N_], f32, tag="cosA")
            nc.sync.dma_start(out=cosA_sb, in_=t["cosA"])
            sinS_sb = wp.tile([P, N_], f32, tag="sinS")
            nc.sync.dma_start(out=sinS_sb, in_=t["sinS"])

            projv_sb = wp.tile([P, N_], fmm, tag="projv")

            for nm, dst, rot in (("q", qT, True), ("k", kT, True), ("v", projv_sb, False)):
                xT_ap = t[f"x{nm}T"]
                w_sb = w_sbs[nm]
                for tg in range(TG):
                    ps = ppp.tile([P, 512], f32, tag="projps")
                    for kt in range(DMK):
                        x_sb = xp.tile([P, 512], fmm, tag="xin")
                        nc.sync.dma_start(
                            out=x_sb,
                            in_=xT_ap[kt * P:(kt + 1) * P, tg * 512:(tg + 1) * 512],
                        )
                        nc.tensor.matmul(
                            ps, w_sb[:, kt, :], x_sb,
                            start=(kt == 0), stop=(kt == DMK - 1),
                        )
                    sl = slice(tg * 512, (tg + 1) * 512)
                    # psum -> sbuf copy, adding the projection bias
                    nc.scalar.activation(dst[:, sl], ps, Act.Identity, bias=b_sbs[nm])
                    if rot:
                        # rotary: out = x*cos' + swap(x)*sin'  (swap via PE perm)
                        swp = swp_ps.tile([P, 512], f32, tag="swp")
                        nc.tensor.matmul(swp, pswap_sb, dst[:, sl],
                                         start=True, stop=True)
                        t1 = rp.tile([P, 512], f32, tag="rot1")
                        nc.vector.tensor_tensor(t1, dst[:, sl], cosA_sb[:, sl], Alu.mult)
                        t2 = rp.tile([P, 512], f32, tag="rot2")
                        nc.vector.tensor_tensor(t2, swp, sinS_sb[:, sl], Alu.mult)
                        nc.vector.tensor_tensor(dst[:, sl], t1, t2, Alu.add)

            # v transpose into natural [tok, dims] blocks (f32 transpose mode)
            for b in range(B_):
                for kt in range(KT):
                    col = b * L_ + kt * P
                    tps = vps.tile([P, P], f32, tag="vtps")
                    nc.tensor.transpose(
                        tps, projv_sb[:, col:col + P].bitcast(f32), ident_sb
                    )
                    nc.scalar.copy(vnat[:, (b * KT + kt) * P:(b * KT + kt + 1) * P], tps)

        # ---- Phase D: attention ----
        with (
            tc.tile_pool(name="mpool", bufs=2) as mp,
            tc.tile_pool(name="tpool", bufs=1) as tp,
            tc.tile_pool(name="wkpool", bufs=2) as wk,
            tc.tile_pool(name="probp", bufs=2) as prp,
            tc.tile_pool(name="denp", bufs=8) as dnp,
            tc.tile_pool(name="scps", bufs=1, space="PSUM") as scps,
            tc.tile_pool(name="ptps", bufs=2, space="PSUM") as ptps,
            tc.tile_pool(name="avps", bufs=1, space="PSUM") as avps,
        ):
            for b in range(B_):
                for qg in range(QG):
                    mneg_sb = mp.tile([P, QT_G, L_], bf16, tag="mneg")
                    nc.sync.dma_start(
                        out=mneg_sb,
                        in_=t["mneg"][b, qg * QT_G * P:(qg + 1) * QT_G * P, :]
                        .rearrange("(t p) k -> p t k", p=P),
                    )
                    for h in range(H_LOC):
                        hd = slice(h * DK, (h + 1) * DK)
                        pT_sb = tp.tile([P, KT, QT_G * P], fmm, tag="pT")
                        for qt in range(QT_G):
                            q0 = b * L_ + (qg * QT_G + qt) * P
                            sps = scps.tile([P, L_], f32, tag="scores")
                            for kc in range(KC):
                                nc.tensor.matmul(
                                    sps[:, kc * KCW:(kc + 1) * KCW],
                                    qT[hd, q0:q0 + P],
                                    kT[hd, b * L_ + kc * KCW: b * L_ + (kc + 1) * KCW],
                                    start=True, stop=True,
                                )
                            sm = wk.tile([P, L_], f32, tag="sm")
                            nc.vector.tensor_tensor(sm, sps, mneg_sb[:, qt, :], Alu.add)
                            e = wk.tile([P, L_], f32, tag="e")
                            den = dnp.tile([P, 1], f32, tag="den")
                            nc.scalar.activation(e, sm, Act.Exp, scale=1.0 / TEMP,
                                                 accum_out=den)
                            rden = dnp.tile([P, 1], f32, tag="rden")
                            nc.vector.reciprocal(rden, den)
                            p = prp.tile([P, L_], f32, tag="p")
                            nc.gpsimd.tensor_scalar_mul(p, e, rden)
                            qrow = (qg * QT_G + qt) * P
                            nc.sync.dma_start(
                                out=t["attn_o"][h, b, qrow:qrow + P, :], in_=p
                            )
                            # transpose p into [k, q] strips for the av matmul
                            for ktg in range((KT + 3) // 4):
                                n_in = min(4, KT - ktg * 4)
                                tps = ptps.tile([P, 512], f32, tag="ptps")
                                for j in range(n_in):
                                    kt = ktg * 4 + j
                                    nc.tensor.transpose(
                                        tps[:, j * P:(j + 1) * P],
                                        p[:, kt * P:(kt + 1) * P],
                                        ident_sb,
                                    )
                                cp_out = pT_sb[:, ktg * 4:ktg * 4 + n_in, qt * P:(qt + 1) * P]
                                cp_in = tps[:, :n_in * P].rearrange("p (t q) -> p t q", q=P)
                                if ktg % 2 == 0:
                                    nc.vector.tensor_copy(out=cp_out, in_=cp_in)
                                else:
                                    nc.scalar.copy(cp_out, cp_in)
                        # av^T = v^T @ p^T for this (h, b, qg)
                        aps = avps.tile([DV, QT_G * P], f32, tag="av")
                        for kt in range(KT):
                            nc.tensor.matmul(
                                aps,
                                vnat[:, (b * KT + kt) * P + h * DK:
                                     (b * KT + kt) * P + (h + 1) * DK],
                                pT_sb[:, kt, :],
                                start=(kt == 0), stop=(kt == KT - 1),
                            )
                        c0 = b * L_ + qg * QT_G * P
                        nc.scalar.copy(avT[hd, c0:c0 + QT_G * P], aps)

        # ---- Phase E: partial fc ----
        with (
            tc.tile_pool(name="ypool", bufs=3) as yp,
            tc.tile_pool(name="fcps", bufs=2, space="PSUM") as fps_pool,
        ):
            for tt in range(N_ // P):
                fps = fps_pool.tile([P, DM], f32, tag="fc")
                for nk in range(DM // 512):
                    nc.tensor.matmul(
                        fps[:, nk * 512:(nk + 1) * 512],
                        avT[:, tt * P:(tt + 1) * P],
                        wfct_sb[:, nk * 512:(nk + 1) * 512],
                        start=True, stop=True,
                    )
                y_sb = yp.tile([P, DM], f32, tag="y")
                nc.scalar.copy(y_sb, fps)
                nc.sync.dma_start(out=t["y_part"][tt * P:(tt + 1) * P, :], in_=y_sb)


def build(L_=L, B_=B, use_f32r=True, compile_=True, loop_n=0):
    """loop_n > 0 wraps the body in a hardware For_i loop (timing builds)."""
    import concourse.bass as bass  # noqa: F401
    from concourse import bacc, mybir, tile

    f32 = mybir.dt.float32
    fmm = mybir.dt.float32r if use_f32r else f32
    N_ = B_ * L_
    nc = bacc.Bacc("TRN2", target_bir_lowering=False, debug=False,
                   num_devices=NCORES)

    t = {}

    def inp(name, shape, dt=f32):
        t[name] = nc.dram_tensor(name, list(shape), dt, kind="ExternalInput").ap()

    def outp(name, shape, dt=f32):
        t[name] = nc.dram_tensor(name, list(shape), dt, kind="ExternalOutput").ap()

    for nm in ("q", "k", "v"):
        inp(f"x{nm}T", (DM, N_), fmm)
        inp(f"w{nm}T", (DM, D_LOC), fmm)
        inp(f"b{nm}", (D_LOC, 1))
    inp("cosA", (D_LOC, N_))
    inp("sinS", (D_LOC, N_))
    inp("mneg", (B_, L_, L_), mybir.dt.bfloat16)
    inp("pswap", (P, P), fmm)
    inp("ident", (P, P))
    inp("wfcT", (D_LOC, DM), fmm)
    outp("attn_o", (H_LOC, B_, L_, L_))
    outp("y_part", (N_, DM))

    with tile.TileContext(nc) as tc:
        if loop_n > 0:
            with tc.For_i(0, loop_n, 1):
                _emit(tc, nc, t, L_, B_, use_f32r=use_f32r)
        else:
            _emit(tc, nc, t, L_, B_, use_f32r=use_f32r)

    if compile_:
        nc.compile()
    return nc


# ----------------------------------------------------------------------------
# Host-side input prep / output assembly
# ----------------------------------------------------------------------------

def _host_prep(q, k, v, mask, w_q, b_q, w_k, b_k, w_v, b_v, w_fc, L_=L, B_=B):
    """Build the 8 per-core input maps."""
    N_ = B_ * L_
    f32 = np.float32

    xqT = np.ascontiguousarray(q.reshape(N_, DM).T.astype(f32, copy=False))
    xkT = np.ascontiguousarray(k.reshape(N_, DM).T.astype(f32, copy=False))
    xvT = np.ascontiguousarray(v.reshape(N_, DM).T.astype(f32, copy=False))
    import ml_dtypes
    mneg = np.where(mask, f32(NEG), f32(0.0)).astype(ml_dtypes.bfloat16)
    mneg = np.ascontiguousarray(mneg)

    pswap = np.zeros((P, P), f32)
    idx = np.arange(0, P, 2)
    pswap[idx, idx + 1] = 1.0
    pswap[idx + 1, idx] = 1.0
    ident = np.eye(P, dtype=f32)

    # rotary tables (full-D angle enc: inv freq indexed over all H*DK/2 pairs)
    D_full = H * DK
    pos = np.tile(np.arange(1, L_ + 1, dtype=f32), B_)          # (N,)

    in_maps = []
    for c in range(NCORES):
        rs = slice(c * D_LOC, (c + 1) * D_LOC)                  # proj dim rows
        j = np.arange(c * (D_LOC // 2), (c + 1) * (D_LOC // 2), dtype=f32)
        inv = np.power(f32(10000.0), -2.0 * j / D_full).astype(f32)
        ang = inv[:, None] * pos[None, :]                       # (64, N)
        cosA = np.repeat(np.cos(ang), 2, axis=0).astype(f32)    # (128, N)
        s = np.sin(ang).astype(f32)
        sinS = np.empty((D_LOC, N_), f32)
        sinS[0::2] = -s
        sinS[1::2] = s
        m = {
            "xqT": xqT, "xkT": xkT, "xvT": xvT, "mneg": mneg,
            "pswap": pswap, "ident": ident,
            "wqT": np.ascontiguousarray(w_q[rs].T.astype(f32, copy=False)),
            "wkT": np.ascontiguousarray(w_k[rs].T.astype(f32, copy=False)),
            "wvT": np.ascontiguousarray(w_v[rs].T.astype(f32, copy=False)),
            "bq": np.ascontiguousarray(b_q[rs].astype(f32, copy=False)).reshape(D_LOC, 1),
            "bk": np.ascontiguousarray(b_k[rs].astype(f32, copy=False)).reshape(D_LOC, 1),
            "bv": np.ascontiguousarray(b_v[rs].astype(f32, copy=False)).reshape(D_LOC, 1),
            "wfcT": np.ascontiguousarray(w_fc[:, rs].T.astype(f32, copy=False)),
            "cosA": np.ascontiguousarray(cosA),
            "sinS": np.ascontiguousarray(sinS),
        }
        in_maps.append(m)
    return in_maps


def _host_finish(results, q, b_fc, gamma, beta, L_=L, B_=B):
    N_ = B_ * L_
    f32 = np.float32
    attn = np.empty((H, B_, L_, L_), f32)
    y = np.zeros((N_, DM), f32)
    for c, res in enumerate(results):
        attn[c * H_LOC:(c + 1) * H_LOC] = res["attn_o"]
        y += res["y_part"]
    y = y + b_fc[None, :].astype(f32) + q.reshape(N_, DM).astype(f32)
    mu = y.mean(axis=-1, keepdims=True, dtype=f32)
    var = y.var(axis=-1, keepdims=True, dtype=f32)
    y = (y - mu) / np.sqrt(var + EPS) * gamma[None, :] + beta[None, :]
    return (y.reshape(B_, L_, DM).astype(f32),
            attn.reshape(H * B_, L_, L_))


def kernel(q, k, v, mask, w_q, b_q, w_k, b_k, w_v, b_v, w_fc, b_fc, gamma, beta):
    from concourse import bass_utils

    key = (L, B, True)
    if key not in _CACHE:
        _CACHE[key] = build(L, B, use_f32r=True)
    nc = _CACHE[key]

    in_maps = _host_prep(q, k, v, mask, w_q, b_q, w_k, b_k, w_v, b_v, w_fc)
    res = bass_utils.run_bass_kernel_spmd(nc, in_maps, core_ids=list(range(NCORES)))
    return _host_finish(res.results, q, b_fc, gamma, beta)


# ----------------------------------------------------------------------------
# Pure-numpy golden model (for self-tests only; harness never calls this)
# ----------------------------------------------------------------------------

def golden(q, k, v, mask, w_q, b_q, w_k, b_k, w_v, b_v, w_fc, b_fc, gamma, beta,
           L_=L, B_=B):
    def angle_enc(x):
        Bx, Lx, D = x.shape
        j = np.arange(D // 2, dtype=np.float64)
        inv = np.power(10000.0, -2.0 * j / D)
        pos = np.arange(1, Lx + 1, dtype=np.float64)
        ang = pos[:, None] * inv[None, :]
        c, s = np.cos(ang), np.sin(ang)
        xp = x.reshape(Bx, Lx, D // 2, 2)
        x0, x1 = xp[..., 0], xp[..., 1]
        out = np.stack([x0 * c - x1 * s, x1 * c + x0 * s], axis=-1)
        return out.reshape(Bx, Lx, D)

    qp = angle_enc(q @ w_q.T + b_q)
    kp = angle_enc(k @ w_k.T + b_k)
    vp = v @ w_v.T + b_v
    qh = qp.reshape(B_, L_, H, DK).transpose(2, 0, 1, 3)
    kh = kp.reshape(B_, L_, H, DK).transpose(2, 0, 1, 3)
    vh = vp.reshape(B_, L_, H, DV).transpose(2, 0, 1, 3)
    scores = np.einsum("hbqd,hbkd->hbqk", qh, kh) / TEMP
    mb = mask[None]
    e = np.exp(scores) * (~mb)
    den = e.sum(-1, keepdims=True)
    attn = e / den
    attn = np.where(mb, 0.0, attn)
    out = np.einsum("hbqk,hbkd->hbqd", attn, vh)
    out = out.transpose(1, 2, 0, 3).reshape(B_, L_, H * DV)
    out = out @ w_fc.T + b_fc
    y = out + q
    mu = y.mean(-1, keepdims=True)
    var = y.var(-1, keepdims=True)
    y = (y - mu) / np.sqrt(var + EPS) * gamma + beta
    return y.astype(np.float32), attn.reshape(H * B_, L_, L_).astype(np.float32)


# revision 17
# speedup vs baseline: 112.7634x; 2.8073x over previous
"""MultiHeadAttention (rotary + masked softmax + fc + residual + layernorm)
Bass/Tile kernel for 8 Trainium2 NeuronCores.

Sharding: tensor-parallel on heads. 16 heads / 8 cores = 2 heads per core.
Each core:
  - projects q/k/v onto its 2 heads' weight slices (contraction over full DM)
  - applies the rotary ("Angle") encoding to its q/k projections
  - computes masked softmax attention for its (2 heads x 2 batches)
  - writes its slice of the [H*B, L, L] attention tensor
  - computes a partial fc output (row-parallel over the 128 av-dims it owns)
Host: transposes inputs once, sums the 8 fc partials, adds bias + residual,
applies the final layernorm.

Matmuls run as float32r (full-rate fp32 streaming on the PE array) when
use_f32r; the attention-probability path (p, its transposes, the attn output)
stays plain f32.
"""

import numpy as np

# Problem dims (hardcoded per the harness contract)
H, DM, DK, DV = 16, 1024, 64, 64
B, L = 2, 2048
TEMP = float(np.sqrt(DK))
EPS = 1e-5
NCORES = 8
H_LOC = H // NCORES          # heads per core
D_LOC = H_LOC * DK           # 128 proj dims per core
P = 128
NEG = -1.0e30

_CACHE = {}


# ----------------------------------------------------------------------------
# Device kernel (per-core SPMD program)
# ----------------------------------------------------------------------------

def _emit(tc, nc, t, L_, B_, use_f32r=True):
    """Emit the per-core program. `t` maps name -> dram AP."""
    from concourse import mybir

    f32 = mybir.dt.float32
    bf16 = mybir.dt.bfloat16
    fmm = mybir.dt.float32r if use_f32r else f32
    Alu = mybir.AluOpType
    Act = mybir.ActivationFunctionType

    N_ = B_ * L_                 # total tokens
    TG = N_ // 512               # 512-token groups for projections
    DMK = DM // P                # 8 k-tiles over the DM contraction
    KT = L_ // P                 # 128-wide k tiles per batch
    QG = max(L_ // 512, 1)       # 512-row q groups per batch
    QT_G = min(L_, 512) // P     # q tiles per group
    KC = max(L_ // 512, 1)       # 512-wide k chunks
    KCW = min(L_, 512)           # k chunk width

    with tc.tile_pool(name="persist", bufs=1) as persist:
        qT = persist.tile([P, N_], fmm)      # rotated q projection [dims, tok]
        kT = persist.tile([P, N_], fmm)      # rotated k projection [dims, tok]
        vnat = persist.tile([P, N_], fmm)    # v proj, natural: (b,kt) blocks of [tok128, dims128]
        avT = persist.tile([P, N_], fmm)     # attention output [dims, tok]
        wfct_sb = persist.tile([P, DM], fmm)
        ident_sb = persist.tile([P, P], f32)
        nc.sync.dma_start(out=wfct_sb, in_=t["wfcT"])
        nc.sync.dma_start(out=ident_sb, in_=t["ident"])

        # ---- Phase A+B: q/k/v projections, rotary on q/k, v transpose ----
        with (
            tc.tile_pool(name="wpool", bufs=1) as wp,
            tc.tile_pool(name="xpool", bufs=3) as xp,
            tc.tile_pool(name="rotpool", bufs=3) as rp,
            tc.tile_pool(name="ppsum", bufs=2, space="PSUM") as ppp,
            tc.tile_pool(name="spsum", bufs=2, space="PSUM") as swp_ps,
            tc.tile_pool(name="vpsum", bufs=2, space="PSUM") as vps,
        ):
            w_sbs = {}
            b_sbs = {}
            for nm in ("q", "k", "v"):
                w_sb = wp.tile([P, DMK, P], fmm, tag=f"w{nm}")
                nc.sync.dma_start(
                    out=w_sb, in_=t[f"w{nm}T"].rearrange("(t p) d -> p t d", p=P)
                )
                w_sbs[nm] = w_sb
                b_sb = wp.tile([P, 1], f32, tag=f"b{nm}")
                nc.sync.dma_start(out=b_sb, in_=t[f"b{nm}"])
                b_sbs[nm] = b_sb
            pswap_sb = wp.tile([P, P], fmm, tag="pswap")
            nc.sync.dma_start(out=pswap_sb, in_=t["pswap"])
            cosA_sb = wp.tile([P, N_], f32, tag="cosA")
            nc.sync.dma_start(out=cosA_sb, in_=t["cosA"])
            sinS_sb = wp.tile([P, N_], f32, tag="sinS")
            nc.sync.dma_start(out=sinS_sb, in_=t["sinS"])

            projv_sb = wp.tile([P, N_], fmm, tag="projv")

            for nm, dst, rot in (("q", qT, True), ("k", kT, True), ("v", projv_sb, False)):
                xT_ap = t[f"x{nm}T"]
                w_sb = w_sbs[nm]
                for tg in range(TG):
                    ps = ppp.tile([P, 512], f32, tag="projps")
                    for kt in range(DMK):
                        x_sb = xp.tile([P, 512], fmm, tag="xin")
                        nc.sync.dma_start(
                            out=x_sb,
                            in_=xT_ap[kt * P:(kt + 1) * P, tg * 512:(tg + 1) * 512],
                        )
                        nc.tensor.matmul(
                            ps, w_sb[:, kt, :], x_sb,
                            start=(kt == 0), stop=(kt == DMK - 1),
                        )
                    sl = slice(tg * 512, (tg + 1) * 512)
                    # psum -> sbuf copy, adding the projection bias
                    nc.scalar.activation(dst[:, sl], ps, Act.Identity, bias=b_sbs[nm])
                    if rot:
                        # rotary: out = x*cos' + swap(x)*sin'  (swap via PE perm)
                        swp = swp_ps.tile([P, 512], f32, tag="swp")
                        nc.tensor.matmul(swp, pswap_sb, dst[:, sl],
                                         start=True, stop=True)
                        t1 = rp.tile([P, 512], f32, tag="rot1")
                        nc.vector.tensor_tensor(t1, dst[:, sl], cosA_sb[:, sl], Alu.mult)
                        t2 = rp.tile([P, 512], f32, tag="rot2")
                        nc.vector.tensor_tensor(t2, swp, sinS_sb[:, sl], Alu.mult)
                        nc.vector.tensor_tensor(dst[:, sl], t1, t2, Alu.add)

            # v transpose into natural [tok, dims] blocks (f32 transpose mode)
            for b in range(B_):
                for kt in range(KT):
                    col = b * L_ + kt * P
                    tps = vps.tile([P, P], f32, tag="vtps")
                    nc.tensor.transpose(
                        tps, projv_sb[:, col:col + P].bitcast(f32), ident_sb
                    )
                    nc.scalar.copy(vnat[:, (b * KT + kt) * P:(b * KT + kt + 1) * P], tps)

        # ---- Phase D: attention ----
        with (
            tc.tile_pool(name="mpool", bufs=2) as mp,
            tc.tile_pool(name="tpool", bufs=1) as tp,
            tc.tile_pool(name="wkpool", bufs=2) as wk,
            tc.tile_pool(name="probp", bufs=2) as prp,
            tc.tile_pool(name="denp", bufs=8) as dnp,
            tc.tile_pool(name="scps", bufs=1, space="PSUM") as scps,
            tc.tile_pool(name="ptps", bufs=2, space="PSUM") as ptps,
            tc.tile_pool(name="avps", bufs=1, space="PSUM") as avps,
        ):
            for b in range(B_):
                for qg in range(QG):
                    mneg_sb = mp.tile([P, QT_G, L_], bf16, tag="mneg")
                    nc.sync.dma_start(
                        out=mneg_sb,
                        in_=t["mneg"][b, qg * QT_G * P:(qg + 1) * QT_G * P, :]
                        .rearrange("(t p) k -> p t k", p=P),
                    )
                    for h in range(H_LOC):
                        hd = slice(h * DK, (h + 1) * DK)
                        pT_sb = tp.tile([P, KT, QT_G * P], fmm, tag="pT")
                        for qt in range(QT_G):
                            q0 = b * L_ + (qg * QT_G + qt) * P
                            sps = scps.tile([P, L_], f32, tag="scores")
                            for kc in range(KC):
                                nc.tensor.matmul(
                                    sps[:, kc * KCW:(kc + 1) * KCW],
                                    qT[hd, q0:q0 + P],
                                    kT[hd, b * L_ + kc * KCW: b * L_ + (kc + 1) * KCW],
                                    start=True, stop=True,
                                )
                            sm = wk.tile([P, L_], f32, tag="sm")
                            nc.vector.tensor_tensor(sm, sps, mneg_sb[:, qt, :], Alu.add)
                            e = wk.tile([P, L_], f32, tag="e")
                            den = dnp.tile([P, 1], f32, tag="den")
                            nc.scalar.activation(e, sm, Act.Exp, scale=1.0 / TEMP,
                                                 accum_out=den)
                            rden = dnp.tile([P, 1], f32, tag="rden")
                            nc.vector.reciprocal(rden, den)
                            p = prp.tile([P, L_], f32, tag="p")
                            nc.gpsimd.tensor_scalar_mul(p, e, rden)
                            qrow = (qg * QT_G + qt) * P
                            nc.sync.dma_start(
                                out=t["attn_o"][h, b, qrow:qrow + P, :], in_=p
                            )
                            # transpose p into [k, q] strips for the av matmul
                            for ktg in range((KT + 3) // 4):
                                n_in = min(4, KT - ktg * 4)
                                tps = ptps.tile([P, 512], f32, tag="ptps")
                                for j in range(n_in):
                                    kt = ktg * 4 + j
                                    nc.tensor.transpose(
                                        tps[:, j * P:(j + 1) * P],
                                        p[:, kt * P:(kt + 1) * P],
                                        ident_sb,
                                    )
                                # keep ACT exclusively on Exp here: any scalar-
                                # engine copy interleaved with Exp reloads the
                                # ACT function table (~µs per switch on HW)
                                nc.vector.tensor_copy(
                                    out=pT_sb[:, ktg * 4:ktg * 4 + n_in, qt * P:(qt + 1) * P],
                                    in_=tps[:, :n_in * P].rearrange("p (t q) -> p t q", q=P),
                                )
                        # av^T = v^T @ p^T for this (h, b, qg)
                        aps = avps.tile([DV, QT_G * P], f32, tag="av")
                        for kt in range(KT):
                            nc.tensor.matmul(
                                aps,
                                vnat[:, (b * KT + kt) * P + h * DK:
                                     (b * KT + kt) * P + (h + 1) * DK],
                                pT_sb[:, kt, :],
                                start=(kt == 0), stop=(kt == KT - 1),
                            )
                        c0 = b * L_ + qg * QT_G * P
                        nc.vector.tensor_copy(out=avT[hd, c0:c0 + QT_G * P], in_=aps)

        # ---- Phase E: partial fc ----
        with (
            tc.tile_pool(name="ypool", bufs=3) as yp,
            tc.tile_pool(name="fcps", bufs=2, space="PSUM") as fps_pool,
        ):
            for tt in range(N_ // P):
                fps = fps_pool.tile([P, DM], f32, tag="fc")
                for nk in range(DM // 512):
                    nc.tensor.matmul(
                        fps[:, nk * 512:(nk + 1) * 512],
                        avT[:, tt * P:(tt + 1) * P],
                        wfct_sb[:, nk * 512:(nk + 1) * 512],
                        start=True, stop=True,
                    )
                y_sb = yp.tile([P, DM], f32, tag="y")
                nc.scalar.copy(y_sb, fps)
                nc.sync.dma_start(out=t["y_part"][tt * P:(tt + 1) * P, :], in_=y_sb)


def build(L_=L, B_=B, use_f32r=True, compile_=True, loop_n=0):
    """loop_n > 0 wraps the body in a hardware For_i loop (timing builds)."""
    import concourse.bass as bass  # noqa: F401
    from concourse import bacc, mybir, tile

    f32 = mybir.dt.float32
    fmm = mybir.dt.float32r if use_f32r else f32
    N_ = B_ * L_
    nc = bacc.Bacc("TRN2", target_bir_lowering=False, debug=False,
                   num_devices=NCORES)

    t = {}

    def inp(name, shape, dt=f32):
        t[name] = nc.dram_tensor(name, list(shape), dt, kind="ExternalInput").ap()

    def outp(name, shape, dt=f32):
        t[name] = nc.dram_tensor(name, list(shape), dt, kind="ExternalOutput").ap()

    for nm in ("q", "k", "v"):
        inp(f"x{nm}T", (DM, N_), fmm)
        inp(f"w{nm}T", (DM, D_LOC), fmm)
        inp(f"b{nm}", (D_LOC, 1))
    inp("cosA", (D_LOC, N_))
    inp("sinS", (D_LOC, N_))
    inp("mneg", (B_, L_, L_), mybir.dt.bfloat16)
    inp("pswap", (P, P), fmm)
    inp("ident", (P, P))
    inp("wfcT", (D_LOC, DM), fmm)
    outp("attn_o", (H_LOC, B_, L_, L_))
    outp("y_part", (N_, DM))

    with tile.TileContext(nc) as tc:
        if loop_n > 0:
            with tc.For_i(0, loop_n, 1):
                _emit(tc, nc, t, L_, B_, use_f32r=use_f32r)
        else:
            _emit(tc, nc, t, L_, B_, use_f32r=use_f32r)

    if compile_:
        nc.compile()
    return nc


# ----------------------------------------------------------------------------
# Host-side input prep / output assembly
# ----------------------------------------------------------------------------

def _host_prep(q, k, v, mask, w_q, b_q, w_k, b_k, w_v, b_v, w_fc, L_=L, B_=B):
    """Build the 8 per-core input maps."""
    N_ = B_ * L_
    f32 = np.float32

    xqT = np.ascontiguousarray(q.reshape(N_, DM).T.astype(f32, copy=False))
    xkT = np.ascontiguousarray(k.reshape(N_, DM).T.astype(f32, copy=False))
    xvT = np.ascontiguousarray(v.reshape(N_, DM).T.astype(f32, copy=False))
    import ml_dtypes
    mneg = np.where(mask, f32(NEG), f32(0.0)).astype(ml_dtypes.bfloat16)
    mneg = np.ascontiguousarray(mneg)

    pswap = np.zeros((P, P), f32)
    idx = np.arange(0, P, 2)
    pswap[idx, idx + 1] = 1.0
    pswap[idx + 1, idx] = 1.0
    ident = np.eye(P, dtype=f32)

    # rotary tables (full-D angle enc: inv freq indexed over all H*DK/2 pairs)
    D_full = H * DK
    pos = np.tile(np.arange(1, L_ + 1, dtype=f32), B_)          # (N,)

    in_maps = []
    for c in range(NCORES):
        rs = slice(c * D_LOC, (c + 1) * D_LOC)                  # proj dim rows
        j = np.arange(c * (D_LOC // 2), (c + 1) * (D_LOC // 2), dtype=f32)
        inv = np.power(f32(10000.0), -2.0 * j / D_full).astype(f32)
        ang = inv[:, None] * pos[None, :]                       # (64, N)
        cosA = np.repeat(np.cos(ang), 2, axis=0).astype(f32)    # (128, N)
        s = np.sin(ang).astype(f32)
        sinS = np.empty((D_LOC, N_), f32)
        sinS[0::2] = -s
        sinS[1::2] = s
        m = {
            "xqT": xqT, "xkT": xkT, "xvT": xvT, "mneg": mneg,
            "pswap": pswap, "ident": ident,
            "wqT": np.ascontiguousarray(w_q[rs].T.astype(f32, copy=False)),
            "wkT": np.ascontiguousarray(w_k[rs].T.astype(f32, copy=False)),
            "wvT": np.ascontiguousarray(w_v[rs].T.astype(f32, copy=False)),
            "bq": np.ascontiguousarray(b_q[rs].astype(f32, copy=False)).reshape(D_LOC, 1),
            "bk": np.ascontiguousarray(b_k[rs].astype(f32, copy=False)).reshape(D_LOC, 1),
            "bv": np.ascontiguousarray(b_v[rs].astype(f32, copy=False)).reshape(D_LOC, 1),
            "wfcT": np.ascontiguousarray(w_fc[:, rs].T.astype(f32, copy=False)),
            "cosA": np.ascontiguousarray(cosA),
            "sinS": np.ascontiguousarray(sinS),
        }
        in_maps.append(m)
    return in_maps


def _host_finish(results, q, b_fc, gamma, beta, L_=L, B_=B):
    N_ = B_ * L_
    f32 = np.float32
    attn = np.empty((H, B_, L_, L_), f32)
    y = np.zeros((N_, DM), f32)
    for c, res in enumerate(results):
        attn[c * H_LOC:(c + 1) * H_LOC] = res["attn_o"]
        y += res["y_part"]
    y = y + b_fc[None, :].astype(f32) + q.reshape(N_, DM).astype(f32)
    mu = y.mean(axis=-1, keepdims=True, dtype=f32)
    var = y.var(axis=-1, keepdims=True, dtype=f32)
    y = (y - mu) / np.sqrt(var + EPS) * gamma[None, :] + beta[None, :]
    return (y.reshape(B_, L_, DM).astype(f32),
            attn.reshape(H * B_, L_, L_))


def kernel(q, k, v, mask, w_q, b_q, w_k, b_k, w_v, b_v, w_fc, b_fc, gamma, beta):
    from concourse import bass_utils

    key = (L, B, True)
    if key not in _CACHE:
        _CACHE[key] = build(L, B, use_f32r=True)
    nc = _CACHE[key]

    in_maps = _host_prep(q, k, v, mask, w_q, b_q, w_k, b_k, w_v, b_v, w_fc)
    res = bass_utils.run_bass_kernel_spmd(nc, in_maps, core_ids=list(range(NCORES)))
    return _host_finish(res.results, q, b_fc, gamma, beta)


# ----------------------------------------------------------------------------
# Pure-numpy golden model (for self-tests only; harness never calls this)
# ----------------------------------------------------------------------------

def golden(q, k, v, mask, w_q, b_q, w_k, b_k, w_v, b_v, w_fc, b_fc, gamma, beta,
           L_=L, B_=B):
    def angle_enc(x):
        Bx, Lx, D = x.shape
        j = np.arange(D // 2, dtype=np.float64)
        inv = np.power(10000.0, -2.0 * j / D)
        pos = np.arange(1, Lx + 1, dtype=np.float64)
        ang = pos[:, None] * inv[None, :]
        c, s = np.cos(ang), np.sin(ang)
        xp = x.reshape(Bx, Lx, D // 2, 2)
        x0, x1 = xp[..., 0], xp[..., 1]
        out = np.stack([x0 * c - x1 * s, x1 * c + x0 * s], axis=-1)
        return out.reshape(Bx, Lx, D)

    qp = angle_enc(q @ w_q.T + b_q)
    kp = angle_enc(k @ w_k.T + b_k)
    vp = v @ w_v.T + b_v
    qh = qp.reshape(B_, L_, H, DK).transpose(2, 0, 1, 3)
    kh = kp.reshape(B_, L_, H, DK).transpose(2, 0, 1, 3)
    vh = vp.reshape(B_, L_, H, DV).transpose(2, 0, 1, 3)
    scores = np.einsum("hbqd,hbkd->hbqk", qh, kh) / TEMP
    mb = mask[None]
    e = np.exp(scores) * (~mb)
    den = e.sum(-1, keepdims=True)
    attn = e / den
    attn = np.where(mb, 0.0, attn)
    out = np.einsum("hbqk,hbkd->hbqd", attn, vh)
    out = out.transpose(1, 2, 0, 3).reshape(B_, L_, H * DV)
    out = out @ w_fc.T + b_fc
    y = out + q
    mu = y.mean(-1, keepdims=True)
    var = y.var(-1, keepdims=True)
    y = (y - mu) / np.sqrt(var + EPS) * gamma + beta
    return y.astype(np.float32), attn.reshape(H * B_, L_, L_).astype(np.float32)
